# revision 57
# baseline (speedup 1.0000x reference)
"""Trainium2 Bass kernel for nn_CLFormer (3-block linear-attention transformer).

Sharding: pure data parallel — batch 32 split as 4 per NeuronCore across 8
cores; all parameters replicated; outputs concatenated.

Per-core layout: 4 batches x 32 channels packed onto the 128 SBUF partitions
("channel-major" [128=4bx32c, L]) for the FC1 side; a token-major copy
([128=tok, (chunk, bc)]) feeds the kv-gram and the k-softmax denominator.

v2 restructure (ACT-bound design):
- FC2 emits token-major output directly: z2 = a1_chunk^T @ W2 (activation
  tile stationary, weight streamed), so gelu2 writes the next block's
  token-major h tiles straight from PSUM — no ingest transposes at all.
  The per-channel bias b2 (free-dim in token-major layout) is preloaded
  into PSUM via a K=1 rank-1 matmul (ones-row x b2-row).
- The last block keeps the channel-major z2 form (ACT bias + accum_out
  pooling works per-partition there, and no ingest is needed).
- Phase 2 ACT ping-pong removed: gelu1(t+1) is emitted before gelu2(t)
  with double-buffered z1 PSUM, so ACT never waits on the z2 matmuls.
- q transposes stay on PE (xbar DMA transpose measured 1.24us per 128x128
  chunk, engine-serialized — unusable in bulk).
All weights packed host-side (pre-replicated, pre-block-diagonalized, BN
folded, bf16 pre-rounded) into two tensors so startup is 2 DMAs.
"""
import sys
import numpy as np

for _p in ("/opt/trn_rl_repo", "/root/.axon_site/_ro/trn_rl_repo"):
    if _p not in sys.path:
        sys.path.append(_p)

from contextlib import ExitStack

import concourse.bass as bass
import concourse.mybir as mybir
import bass_rust
from concourse import tile
from concourse.masks import make_identity
from concourse.bass_utils import run_bass_kernel_spmd

F32 = mybir.dt.float32
BF16 = mybir.dt.bfloat16
U32 = mybir.dt.uint32
AF = mybir.ActivationFunctionType
MUL = mybir.AluOpType.mult
ADD = mybir.AluOpType.add

P = 128
B_LOC = 4            # batches per core
C = 32               # channels
L = 16384            # sequence length
NB = 3               # transformer blocks
DOUT = 10
HEADS = 4
DH = 8
BN_EPS = 1e-5

SLC = 4096           # slice width (tokens per q-glue slice)
NSL = L // SLC       # 4 slices
HLF = 2048           # half-slice (exp/gram granularity)
NHF = L // HLF       # 8 halves per block
NCH = SLC // 128     # 32 chunks per slice
EXT = 144            # chunk pitch in ones-extended token-major tiles
NTP = L // 1024      # 16 phase-2 tiles per block

# host-packed weight layouts
# bf16 pack (columns):
#   [0,384)      W1bd blocks 0..2 (128 cols each)
#   [384,768)    W2bd blocks 0..2
#   [768,896)    headmask
#   [896,897)    ones col | [897,898) pad
#   [898,1026)   ones ROW (partition 0 only, 128 cols)
#   [1026,1538)  b2row4 block 0 (partition 0 only, 512 cols = b2 pattern x4)
#   [1538,2050)  b2row4 block 1
NBF = 2050           # bf16 cols (even)
# f32 pack (columns):
#   0..2 b1 | 3..5 b2 | 6 svecL | 7 tvec | 8..39 Whrep | 40..49 Wfrep | 50 bf
NF32 = 51


# ---------------------------------------------------------------- waitfix --
_WF_SKIP = {"InstEventSemaphore"}
_wf_ctr = [0]


def _fix_sync_waits(nc):
    """Hoist excess sync waits onto InstEventSemaphore (this walrus build
    accepts only 1 wait per instruction). The event-sem executes on the same
    engine stream immediately before, preserving semantics."""
    for fn in nc.m.functions:
        new_blocks = []
        for blk in fn.blocks:
            out = []
            for ins in blk.instructions:
                tname = type(ins).__name__
                si = ins.sync_info
                if si is None or tname in _WF_SKIP:
                    out.append(ins)
                    continue
                waits = list(si.on_wait)
                if len(waits) <= 1:
                    out.append(ins)
                    continue
                keep = waits[-1:]
                excess = waits[:-1]
                for i in range(0, len(excess), 2):
                    chunk = excess[i:i + 2]
                    _wf_ctr[0] += 1
                    ev = mybir.InstEventSemaphore(
                        name=f"wfix{_wf_ctr[0]}", ins=[], outs=[])
                    ev.engine = ins.engine
                    ev.sync_info = mybir.SyncInfo(on_wait=chunk, on_update=[])
                    out.append(ev)
                ins.sync_info = mybir.SyncInfo(
                    on_wait=keep, on_update=list(si.on_update))
                out.append(ins)
            nb = bass_rust.BasicBlock(name=blk.name, instructions=out)
            new_blocks.append(nb)
        fn.blocks = new_blocks


# ---------------------------------------------------------------- program --
def build_program():
    nc = bass.Bass()

    x_d = nc.declare_dram_parameter("x", [B_LOC, C, L], F32, isOutput=False)
    wbf_d = nc.declare_dram_parameter("wbf", [P, NBF // 2], U32, isOutput=False)
    wf32_d = nc.declare_dram_parameter("wf32", [P, NF32], F32, isOutput=False)
    out_d = nc.declare_dram_parameter("out", [B_LOC, DOUT], F32, isOutput=True)

    with ExitStack() as ctx:
        tc = ctx.enter_context(tile.TileContext(nc))
        cst = ctx.enter_context(tc.tile_pool(name="cst", bufs=1))
        xbp = ctx.enter_context(tc.tile_pool(name="xbp", bufs=6))
        hex_ = ctx.enter_context(tc.tile_pool(name="hex", bufs=10))
        etm = ctx.enter_context(tc.tile_pool(name="etm", bufs=8))
        qtm = ctx.enter_context(tc.tile_pool(name="qtm", bufs=6))
        sqp = ctx.enter_context(tc.tile_pool(name="sqp", bufs=8))
        bigq = ctx.enter_context(tc.tile_pool(name="bigq", bufs=1))
        a1p = ctx.enter_context(tc.tile_pool(name="a1p", bufs=5))
        smal = ctx.enter_context(tc.tile_pool(name="smal", bufs=2))
        z1ps = ctx.enter_context(tc.tile_pool(name="z1ps", bufs=2, space="PSUM"))
        z2ps = ctx.enter_context(tc.tile_pool(name="z2ps", bufs=2, space="PSUM"))
        qps = ctx.enter_context(tc.tile_pool(name="qps", bufs=2, space="PSUM"))

        # ---- weights: two packed DMAs (on ACT's DGE; ACT idle early) ---
        wbfu = cst.tile([P, NBF // 2], U32)
        nc.scalar.dma_start(wbfu[:], wbf_d[:])
        wf32 = cst.tile([P, NF32], F32)
        nc.scalar.dma_start(wf32[:], wf32_d[:])
        wbf = wbfu[:].bitcast(BF16)

        W1bd = [wbf[:, 128 * i:128 * (i + 1)] for i in range(NB)]
        W2bd = [wbf[:, 384 + 128 * i:384 + 128 * (i + 1)] for i in range(NB)]
        headmask = wbf[:, 768:896]
        onesrow = wbf[0:1, 898:1026]
        b2row4 = [wbf[0:1, 1026 + 512 * i:1026 + 512 * (i + 1)]
                  for i in range(NB - 1)]
        b1rep = [wf32[:, i:i + 1] for i in range(NB)]
        b2rep = [wf32[:, 3 + i:4 + i] for i in range(NB)]
        svecL = wf32[:, 6:7]
        tvec = wf32[:, 7:8]
        Whrep = wf32[:, 8:40]
        Wfrep = wf32[:, 40:50]
        bf_s = wf32[:, 50:51]

        ident = cst.tile([P, P], BF16)
        make_identity(nc, ident[:])

        pooled_parts = cst.tile([P, 2 * NTP], F32)

        x_cm = x_d[:].rearrange("b c l -> (b c) l")

        def new_he_half(bi, h):
            """Token-major half-slice tile [128, 16*144] bf16 with ones col
            preset at local col 128 of each chunk."""
            he = hex_.tile([P, (HLF // 128) * EXT], BF16, tag="hex",
                           name=f"he{bi}_{h}")
            hv = he[:].rearrange("p (c l) -> p c l", l=EXT)
            nc.vector.memset(hv[:, :, 128:129], 1.0)
            return he

        # ---------------- block-0 ingest: SWDGE cast DMA -> PE transpose --
        # (cast happens in the DMA datapath; qp tiles from the
        # otherwise-idle z1ps pool for 2-buf pipelining)
        h_halves = []
        qp_halves = {}

        def ingest_half(t8):
            xb = xbp.tile([P, HLF], BF16, tag="xb")
            nc.gpsimd.dma_start(xb[:], x_cm[:, HLF * t8:HLF * (t8 + 1)])
            he = new_he_half(0, t8)
            hv = he[:].rearrange("p (c l) -> p c l", l=EXT)
            qps_g = []
            for g in range(2):
                qp = z1ps.tile([P, 1024], BF16, tag="z1")
                for k in range(8):
                    c = 8 * g + k
                    nc.tensor.transpose(
                        qp[:, 128 * k:128 * (k + 1)],
                        xb[:, 128 * c:128 * (c + 1)],
                        ident[:],
                    )
                nc.vector.tensor_copy(
                    hv[:, 8 * g:8 * (g + 1), 0:128],
                    qp[:].rearrange("p (c l) -> p c l", l=128),
                )
                qps_g.append(qp)
            h_halves.append(he)
            qp_halves[t8] = qps_g

        ingest_half(0)
        ingest_half(1)

        for blk in range(NB):
            last = blk == NB - 1
            if blk > 0:
                # scheduler fence: keep block i+1's EXPs (and everything
                # else) from interleaving into block i's phase-2 GELU
                # stream — each EXP<->GELU swap costs 2x1.28us of
                # ACT_TABLE_LOAD
                tc.no_sync_barrier()
            # ======================= pass A: exp + gram =================
            # G_ext[:, 0:128] = gram E^T h ; G_ext[:, 128] = ksum (ones col)
            G_ps = z1ps.tile([P, EXT], F32, tag="z1", name=f"G{blk}")
            q_cm = bigq.tile([P, L], BF16, tag="qcm")

            et_tiles = []
            qt_tiles = {}

            def q_scale(h):
                """qsum -> recip -> q=E*rq (DVE + GPSIMD only; emitted
                early so the serial GPSIMD chain starts right behind the
                exps)."""
                eh = et_tiles[h]
                sq = sqp.tile([P, 256], F32, tag="sq")
                nc.vector.reduce_sum(
                    sq[:],
                    eh[:].rearrange("p (c g d) -> p c g d", g=16, d=DH),
                    axis=mybir.AxisListType.X,
                )
                rq = sqp.tile([P, 256], F32, tag="rq")
                nc.vector.reciprocal_approx_fast(rq[:], sq[:])
                qt = qtm.tile([P, HLF], BF16, tag="qtm")
                nc.gpsimd.tensor_tensor(
                    qt[:].rearrange("p (c g d) -> p c g d", g=16, d=DH),
                    eh[:].rearrange("p (c g d) -> p c g d", g=16, d=DH),
                    rq[:].rearrange("p (c g) -> p c g", g=16)
                        .unsqueeze(-1).broadcast_to([P, 16, 16, DH]),
                    op=MUL,
                )
                qt_tiles[h] = qt

            def q_xpose(h):
                """PE transpose of qt half h into q_cm (emitted where the
                PE queue has slack)."""
                qt = qt_tiles[h]
                for g in range(2):
                    qp = qps.tile([P, 1024], BF16, tag="qp")
                    for k in range(8):
                        c = 8 * g + k
                        nc.tensor.transpose(
                            qp[:, 128 * k:128 * (k + 1)],
                            qt[:, 128 * c:128 * (c + 1)],
                            ident[:],
                        )
                    nc.vector.tensor_copy(
                        q_cm[:, HLF * h + 1024 * g: HLF * h + 1024 * (g + 1)],
                        qp[:],
                    )

            for h in range(NHF):
                he = h_halves[h]
                hv = he[:].rearrange("p (c l) -> p c l", l=EXT)
                et = etm.tile([P, HLF], BF16, tag="etm")
                etv = et[:].rearrange("p (c l) -> p c l", l=128)
                if h == NHF - 1:
                    # split the last exp: halves the gram tail on the
                    # boundary critical path
                    nc.scalar.activation(etv[:, 0:8], hv[:, 0:8, 0:128],
                                         AF.Exp)
                    nc.scalar.activation(etv[:, 8:16], hv[:, 8:16, 0:128],
                                         AF.Exp)
                else:
                    nc.scalar.activation(etv, hv[:, :, 0:128], AF.Exp)
                et_tiles.append(et)
                for c in range(HLF // 128):
                    nc.tensor.matmul(
                        G_ps[:],
                        et[:, 128 * c:128 * (c + 1)],
                        he[:, EXT * c:EXT * (c + 1)],
                        start=(h == 0 and c == 0),
                        stop=(h == NHF - 1 and c == HLF // 128 - 1),
                    )
                if blk == 0 and h + 2 < NHF:
                    ingest_half(h + 2)
                # early q-glue: the GPSIMD scale chain starts as soon as
                # exps land; transposes placed where PE has slack
                if h == 3:
                    q_scale(0)
                    q_xpose(0)

            # ======================= M1 build (before q glue: keeps the
            # M1u matmul ahead of the q transposes in the PE queue; high
            # priority so the DVE steps jump the q-glue queue) ===========
            with tc.high_priority():
                ksC = smal.tile([P, 1], F32, tag="ksC")
                nc.vector.reciprocal(ksC[:], G_ps[:, 128:129])
                G_sb = smal.tile([P, P], BF16, tag="Gsb")
                nc.vector.tensor_tensor(G_sb[:], G_ps[:, 0:128], headmask,
                                        op=MUL)
                # block-diag transpose: DVE 32x32 block transpose (off-diag
                # blocks are zero)
                GT_sb = smal.tile([P, P], BF16, tag="gtsb")
                nc.vector.transpose(GT_sb[:], G_sb[:])
                M1u_t = z1ps.tile([P, P], F32, tag="z1", name=f"M1u{blk}")
                nc.tensor.matmul(M1u_t[:], GT_sb[:], W1bd[blk])
                M1 = smal.tile([P, P], BF16, tag="m1")
                nc.vector.tensor_scalar_mul(M1[:], M1u_t[:], ksC[:])

            # ======================= phase 2 ============================
            he_next = ([new_he_half(blk + 1, h) for h in range(NHF)]
                       if not last else None)

            def z1_mm(t):
                z1 = z1ps.tile([P, 1024], F32, tag="z1", name=f"z1_{blk}_{t}")
                for hh in range(2):
                    nc.tensor.matmul(
                        z1[:, 512 * hh:512 * (hh + 1)], M1[:],
                        q_cm[:, 1024 * t + 512 * hh:1024 * t + 512 * (hh + 1)])
                return z1

            def g1_act(t, z1):
                a1 = a1p.tile([P, 1024], BF16, tag="a1")
                nc.scalar.activation(a1[:], z1[:], AF.Gelu, bias=b1rep[blk])
                return a1

            def z2_mm(t, a1):
                # two [128,512] halves, 2-buf: PE fills half h+1 while the
                # ACT gelu reads half h — kills the serial z2->g2 chain
                halves = []
                for hh in range(2):
                    z2 = z2ps.tile([P, 512], F32, tag="z2",
                                   name=f"z2_{blk}_{t}_{hh}")
                    if last:
                        nc.tensor.matmul(
                            z2[:], W2bd[blk],
                            a1[:, 512 * hh:512 * (hh + 1)])
                    else:
                        # bias preload (K=1 rank-1) + a1-stationary chunks
                        nc.tensor.matmul(
                            z2[:], onesrow, b2row4[blk],
                            start=True, stop=False)
                        for cc in range(4):
                            cch = 4 * hh + cc
                            nc.tensor.matmul(
                                z2[:, 128 * cc:128 * (cc + 1)],
                                a1[:, 128 * cch:128 * (cch + 1)],
                                W2bd[blk],
                                start=False, stop=(cc == 3))
                    halves.append(z2)
                return halves

            def g2_act(t, z2h):
                for hh, z2 in enumerate(z2h):
                    if last:
                        hn = a1p.tile([P, 512], BF16, tag="a1")
                        nc.scalar.activation(
                            hn[:], z2[:], AF.Gelu, bias=b2rep[blk])
                        nc.vector.reduce_sum(
                            pooled_parts[:, 2 * t + hh:2 * t + hh + 1],
                            hn[:], axis=mybir.AxisListType.X)
                    else:
                        # token-major write into next block's he tiles
                        he = he_next[t // 2]
                        hv = he[:].rearrange("p (c l) -> p c l", l=EXT)
                        cb = 8 * (t % 2) + 4 * hh
                        nc.scalar.activation(
                            hv[:, cb:cb + 4, 0:128],
                            z2[:].rearrange("p (c l) -> p c l", l=128),
                            AF.Gelu)

            # interleaved schedule: q-glue emitted per slice just before the
            # phase-2 tiles that consume it (PE queue is in-order), and
            # z1(t+1)/g1(t+1) emitted before g2(t) so ACT never waits on
            # the z2 matmuls.
            prev = None
            for t in range(NTP):
                z1_t = z1_mm(t)
                a1_t = g1_act(t, z1_t)
                if prev is not None:
                    pt, pa = prev
                    z2_p = z2_mm(pt, pa)
                    g2_act(pt, z2_p)
                prev = (t, a1_t)
                # q-glue for later halves, in PE slack after a z2/g2
                # pair; half h must land before z1(tp 2h)
                if t == 0:
                    q_scale(1)
                    q_xpose(1)
                elif t % 2 == 1 and (t + 3) // 2 < NHF:
                    q_scale((t + 3) // 2)
                    q_xpose((t + 3) // 2)
            pt, pa = prev
            z2_p = z2_mm(pt, pa)
            g2_act(pt, z2_p)

            if not last:
                h_halves = he_next

        # ============================ head ==============================
        psum_ = smal.tile([P, 1], F32, tag="poolsum")
        nc.vector.reduce_sum(psum_[:], pooled_parts[:],
                             axis=mybir.AxisListType.X)
        y_t = z1ps.tile([P, 1], F32, tag="z1", name="yps")
        for b in range(B_LOC):
            sl = slice(C * b, C * (b + 1))
            nc.tensor.matmul(
                y_t[sl, :], Whrep[sl, :], psum_[sl, :],
                tile_position=(C * b, C * b),
            )
        ybn = smal.tile([P, 1], F32, tag="ybn")
        nc.vector.tensor_scalar(
            ybn[:], y_t[:], svecL, tvec, op0=MUL, op1=ADD,
        )
        yg = smal.tile([P, 1], F32, tag="yg")
        nc.scalar.activation(yg[:], ybn[:], AF.Gelu)
        o_t = z1ps.tile([P, 1], F32, tag="z1", name="ops")
        for b in range(B_LOC):
            nc.tensor.matmul(
                o_t[C * b:C * b + DOUT, :],
                Wfrep[C * b:C * (b + 1), :],
                yg[C * b:C * (b + 1), :],
                tile_position=(C * b, C * b),
            )
        ob = smal.tile([P, C], F32, tag="ob")
        for b in range(B_LOC):
            sl = slice(C * b, C * b + DOUT)
            nc.vector.tensor_tensor(ob[sl, 0:1], o_t[sl, :], bf_s[sl, :],
                                    op=ADD)
        obT = smal.tile([P, C], F32, tag="obT")
        nc.vector.transpose(obT[:], ob[:])
        obv = obT[:].rearrange("(b r) d -> b r d", b=B_LOC)[:, 0, 0:DOUT]
        nc.sync.dma_start(out_d[:, :], obv)

    _fix_sync_waits(nc)
    from concourse.library_overlay import lower_extended_insts
    lower_extended_insts(nc)   # encode custom-DVE InstISA bytes
    return nc


# ------------------------------------------------------------- host pack --
def _bf16_bits(a):
    """float32 -> bf16 bit pattern (round to nearest even), as uint16."""
    u = np.ascontiguousarray(a, dtype="<f4").view("<u4")
    r = ((u >> 16) & 1) + np.uint32(0x7FFF)
    return ((u + r) >> 16).astype("<u2")


def _rep4(v):
    """[32]/[32,k] -> [128]/[128,k] replicated across 4 batch strips."""
    return np.concatenate([v] * B_LOC, axis=0)


def pack_weights(a):
    """Build the two packed weight tensors from the raw input dict."""
    bf = np.zeros((P, NBF), np.float32)
    eye4 = np.eye(B_LOC, dtype=np.float32)
    for i in range(NB):
        bf[:, 128 * i:128 * (i + 1)] = np.kron(eye4, a["fcW1"][i])
        bf[:, 384 + 128 * i:384 + 128 * (i + 1)] = np.kron(eye4, a["fcW2"][i])
    idx = np.arange(P)
    bf[:, 768:896] = (idx[:, None] // DH == idx[None, :] // DH)
    bf[:, 896] = 1.0
    bf[0, 898:1026] = 1.0                       # ones row
    for i in range(NB - 1):
        bf[0, 1026 + 512 * i:1026 + 512 * (i + 1)] = np.tile(
            _rep4(a["fcb2"][i]), 4)             # b2 pattern x4 (512 cols)
    bfu = _bf16_bits(bf).view("<u4").reshape(P, NBF // 2)

    f32 = np.zeros((P, NF32), np.float32)
    for i in range(NB):
        f32[:, i] = _rep4(a["fcb1"][i])
        f32[:, 3 + i] = _rep4(a["fcb2"][i])
    svec = a["bn_gamma"] / np.sqrt(a["bn_var"] + BN_EPS)
    f32[:, 6] = _rep4(svec / L)
    f32[:, 7] = _rep4((a["bh"] - a["bn_mean"]) * svec + a["bn_beta"])
    f32[:, 8:40] = _rep4(a["Wh"])
    f32[:, 40:50] = _rep4(a["Wf"])
    bfcol = np.zeros(C, np.float32)
    bfcol[:DOUT] = a["bf"]
    f32[:, 50] = _rep4(bfcol)
    return np.ascontiguousarray(bfu), np.ascontiguousarray(f32)


_NC_CACHE = [None]


def kernel(**inputs) -> np.ndarray:
    arrs = {k: np.asarray(v, dtype=np.float32) for k, v in inputs.items()}
    x = arrs["x"]
    B = x.shape[0]
    n_cores = 8
    bl = B // n_cores

    if _NC_CACHE[0] is None:
        _NC_CACHE[0] = build_program()
    nc = _NC_CACHE[0]

    wbf, wf32 = pack_weights(arrs)
    in_maps = [
        {"x": np.ascontiguousarray(x[bl * i: bl * (i + 1)]),
         "wbf": wbf, "wf32": wf32}
        for i in range(n_cores)
    ]
    res = run_bass_kernel_spmd(nc, in_maps, list(range(n_cores))).results
    return np.concatenate([res[i]["out"] for i in range(n_cores)], axis=0)


# revision 58
# speedup vs baseline: 1.1117x; 1.1117x over previous
"""Trainium2 Bass kernel for nn_CLFormer (3-block linear-attention transformer).

Sharding: pure data parallel — batch 32 split as 4 per NeuronCore across 8
cores; all parameters replicated; outputs concatenated.

Per-core layout: 4 batches x 32 channels packed onto the 128 SBUF partitions
("channel-major" [128=4bx32c, L]) for the FC1 side; a token-major copy
([128=tok, (chunk, bc)]) feeds the kv-gram and the k-softmax denominator.

v2 restructure (ACT-bound design):
- FC2 emits token-major output directly: z2 = a1_chunk^T @ W2 (activation
  tile stationary, weight streamed), so gelu2 writes the next block's
  token-major h tiles straight from PSUM — no ingest transposes at all.
  The per-channel bias b2 (free-dim in token-major layout) is preloaded
  into PSUM via a K=1 rank-1 matmul (ones-row x b2-row).
- The last block keeps the channel-major z2 form (ACT bias + accum_out
  pooling works per-partition there, and no ingest is needed).
- Phase 2 ACT ping-pong removed: gelu1(t+1) is emitted before gelu2(t)
  with double-buffered z1 PSUM, so ACT never waits on the z2 matmuls.
- q transposes stay on PE (xbar DMA transpose measured 1.24us per 128x128
  chunk, engine-serialized — unusable in bulk).
All weights packed host-side (pre-replicated, pre-block-diagonalized, BN
folded, bf16 pre-rounded) into two tensors so startup is 2 DMAs.
"""
import sys
import numpy as np

for _p in ("/opt/trn_rl_repo", "/root/.axon_site/_ro/trn_rl_repo"):
    if _p not in sys.path:
        sys.path.append(_p)

from contextlib import ExitStack

import concourse.bass as bass
import concourse.mybir as mybir
import bass_rust
from concourse import tile
from concourse.masks import make_identity
from concourse.bass_utils import run_bass_kernel_spmd

F32 = mybir.dt.float32
BF16 = mybir.dt.bfloat16
U32 = mybir.dt.uint32
AF = mybir.ActivationFunctionType
MUL = mybir.AluOpType.mult
ADD = mybir.AluOpType.add

P = 128
B_LOC = 4            # batches per core
C = 32               # channels
L = 16384            # sequence length
NB = 3               # transformer blocks
DOUT = 10
HEADS = 4
DH = 8
BN_EPS = 1e-5

SLC = 4096           # slice width (tokens per q-glue slice)
NSL = L // SLC       # 4 slices
HLF = 2048           # half-slice (exp/gram granularity)
NHF = L // HLF       # 8 halves per block
NCH = SLC // 128     # 32 chunks per slice
EXT = 144            # chunk pitch in ones-extended token-major tiles
NTP = L // 1024      # 16 phase-2 tiles per block

# host-packed weight layouts
# bf16 pack (columns):
#   [0,384)      W1bd blocks 0..2 (128 cols each)
#   [384,768)    W2bd blocks 0..2
#   [768,896)    headmask
#   [896,897)    ones col | [897,898) pad
#   [898,1026)   ones ROW (partition 0 only, 128 cols)
#   [1026,1538)  b2row4 block 0 (partition 0 only, 512 cols = b2 pattern x4)
#   [1538,2050)  b2row4 block 1
NBF = 2050           # bf16 cols (even)
# f32 pack (columns):
#   0..2 b1 | 3..5 b2 | 6 svecL | 7 tvec | 8..39 Whrep | 40..49 Wfrep | 50 bf
NF32 = 51


# ---------------------------------------------------------------- waitfix --
_WF_SKIP = {"InstEventSemaphore"}
_wf_ctr = [0]


def _fix_sync_waits(nc):
    """Hoist excess sync waits onto InstEventSemaphore (this walrus build
    accepts only 1 wait per instruction). The event-sem executes on the same
    engine stream immediately before, preserving semantics."""
    for fn in nc.m.functions:
        new_blocks = []
        for blk in fn.blocks:
            out = []
            for ins in blk.instructions:
                tname = type(ins).__name__
                si = ins.sync_info
                if si is None or tname in _WF_SKIP:
                    out.append(ins)
                    continue
                waits = list(si.on_wait)
                if len(waits) <= 1:
                    out.append(ins)
                    continue
                keep = waits[-1:]
                excess = waits[:-1]
                for i in range(0, len(excess), 2):
                    chunk = excess[i:i + 2]
                    _wf_ctr[0] += 1
                    ev = mybir.InstEventSemaphore(
                        name=f"wfix{_wf_ctr[0]}", ins=[], outs=[])
                    ev.engine = ins.engine
                    ev.sync_info = mybir.SyncInfo(on_wait=chunk, on_update=[])
                    out.append(ev)
                ins.sync_info = mybir.SyncInfo(
                    on_wait=keep, on_update=list(si.on_update))
                out.append(ins)
            nb = bass_rust.BasicBlock(name=blk.name, instructions=out)
            new_blocks.append(nb)
        fn.blocks = new_blocks


# ---------------------------------------------------------------- program --
def build_program():
    nc = bass.Bass()

    x_d = nc.declare_dram_parameter("x", [B_LOC, C, L], F32, isOutput=False)
    wbf_d = nc.declare_dram_parameter("wbf", [P, NBF // 2], U32, isOutput=False)
    wf32_d = nc.declare_dram_parameter("wf32", [P, NF32], F32, isOutput=False)
    out_d = nc.declare_dram_parameter("out", [B_LOC, DOUT], F32, isOutput=True)

    with ExitStack() as ctx:
        tc = ctx.enter_context(tile.TileContext(nc))
        cst = ctx.enter_context(tc.tile_pool(name="cst", bufs=1))
        xbp = ctx.enter_context(tc.tile_pool(name="xbp", bufs=6))
        hex_ = ctx.enter_context(tc.tile_pool(name="hex", bufs=10))
        etm = ctx.enter_context(tc.tile_pool(name="etm", bufs=8))
        qtm = ctx.enter_context(tc.tile_pool(name="qtm", bufs=6))
        sqp = ctx.enter_context(tc.tile_pool(name="sqp", bufs=8))
        bigq = ctx.enter_context(tc.tile_pool(name="bigq", bufs=1))
        a1p = ctx.enter_context(tc.tile_pool(name="a1p", bufs=5))
        smal = ctx.enter_context(tc.tile_pool(name="smal", bufs=2))
        gps = ctx.enter_context(tc.tile_pool(name="gps", bufs=1, space="PSUM"))
        z1ps = ctx.enter_context(tc.tile_pool(name="z1ps", bufs=2, space="PSUM"))
        z2ps = ctx.enter_context(tc.tile_pool(name="z2ps", bufs=2, space="PSUM"))
        qps = ctx.enter_context(tc.tile_pool(name="qps", bufs=1, space="PSUM"))

        # ---- weights: two packed DMAs (on ACT's DGE; ACT idle early) ---
        wbfu = cst.tile([P, NBF // 2], U32)
        nc.scalar.dma_start(wbfu[:], wbf_d[:])
        wf32 = cst.tile([P, NF32], F32)
        nc.scalar.dma_start(wf32[:], wf32_d[:])
        wbf = wbfu[:].bitcast(BF16)

        W1bd = [wbf[:, 128 * i:128 * (i + 1)] for i in range(NB)]
        W2bd = [wbf[:, 384 + 128 * i:384 + 128 * (i + 1)] for i in range(NB)]
        headmask = wbf[:, 768:896]
        onesrow = wbf[0:1, 898:1026]
        b2row4 = [wbf[0:1, 1026 + 512 * i:1026 + 512 * (i + 1)]
                  for i in range(NB - 1)]
        b1rep = [wf32[:, i:i + 1] for i in range(NB)]
        b2rep = [wf32[:, 3 + i:4 + i] for i in range(NB)]
        svecL = wf32[:, 6:7]
        tvec = wf32[:, 7:8]
        Whrep = wf32[:, 8:40]
        Wfrep = wf32[:, 40:50]
        bf_s = wf32[:, 50:51]

        ident = cst.tile([P, P], BF16)
        make_identity(nc, ident[:])

        pooled_parts = cst.tile([P, 2 * NTP], F32)

        x_cm = x_d[:].rearrange("b c l -> (b c) l")

        def new_he_half(bi, h):
            """Token-major half-slice tile [128, 16*144] bf16 with ones col
            preset at local col 128 of each chunk."""
            he = hex_.tile([P, (HLF // 128) * EXT], BF16, tag="hex",
                           name=f"he{bi}_{h}")
            hv = he[:].rearrange("p (c l) -> p c l", l=EXT)
            nc.vector.memset(hv[:, :, 128:129], 1.0)
            return he

        # ---------------- block-0 ingest: SWDGE cast DMA -> PE transpose --
        # (cast happens in the DMA datapath; qp tiles from the
        # otherwise-idle z1ps pool for 2-buf pipelining)
        h_halves = []
        qp_halves = {}

        def ingest_half(t8):
            xb = xbp.tile([P, HLF], BF16, tag="xb")
            nc.gpsimd.dma_start(xb[:], x_cm[:, HLF * t8:HLF * (t8 + 1)])
            he = new_he_half(0, t8)
            hv = he[:].rearrange("p (c l) -> p c l", l=EXT)
            qps_g = []
            for g in range(2):
                qp = z1ps.tile([P, 1024], BF16, tag="z1")
                for k in range(8):
                    c = 8 * g + k
                    nc.tensor.transpose(
                        qp[:, 128 * k:128 * (k + 1)],
                        xb[:, 128 * c:128 * (c + 1)],
                        ident[:],
                    )
                nc.vector.tensor_copy(
                    hv[:, 8 * g:8 * (g + 1), 0:128],
                    qp[:].rearrange("p (c l) -> p c l", l=128),
                )
                qps_g.append(qp)
            h_halves.append(he)
            qp_halves[t8] = qps_g

        ingest_half(0)
        ingest_half(1)

        for blk in range(NB):
            last = blk == NB - 1
            if blk > 0:
                # scheduler fence: keep block i+1's EXPs (and everything
                # else) from interleaving into block i's phase-2 GELU
                # stream — each EXP<->GELU swap costs 2x1.28us of
                # ACT_TABLE_LOAD
                tc.no_sync_barrier()
            # ======================= pass A: exp + gram =================
            # G_ext[:, 0:128] = gram E^T h ; G_ext[:, 128] = ksum (ones col)
            G_ps = gps.tile([P, EXT], F32, tag="G")
            q_cm = bigq.tile([P, L], BF16, tag="qcm")

            et_tiles = []
            qt_tiles = {}

            def q_scale(h):
                """qsum -> recip -> q=E*rq (DVE + GPSIMD only; emitted
                early so the serial GPSIMD chain starts right behind the
                exps)."""
                eh = et_tiles[h]
                sq = sqp.tile([P, 256], F32, tag="sq")
                nc.vector.reduce_sum(
                    sq[:],
                    eh[:].rearrange("p (c g d) -> p c g d", g=16, d=DH),
                    axis=mybir.AxisListType.X,
                )
                rq = sqp.tile([P, 256], F32, tag="rq")
                nc.vector.reciprocal_approx_fast(rq[:], sq[:])
                qt = qtm.tile([P, HLF], BF16, tag="qtm")
                nc.gpsimd.tensor_tensor(
                    qt[:].rearrange("p (c g d) -> p c g d", g=16, d=DH),
                    eh[:].rearrange("p (c g d) -> p c g d", g=16, d=DH),
                    rq[:].rearrange("p (c g) -> p c g", g=16)
                        .unsqueeze(-1).broadcast_to([P, 16, 16, DH]),
                    op=MUL,
                )
                qt_tiles[h] = qt

            def q_xpose(h):
                """PE transpose of qt half h into q_cm (emitted where the
                PE queue has slack)."""
                qt = qt_tiles[h]
                for g in range(2):
                    qp = qps.tile([P, 1024], BF16, tag="qp")
                    for k in range(8):
                        c = 8 * g + k
                        nc.tensor.transpose(
                            qp[:, 128 * k:128 * (k + 1)],
                            qt[:, 128 * c:128 * (c + 1)],
                            ident[:],
                        )
                    nc.vector.tensor_copy(
                        q_cm[:, HLF * h + 1024 * g: HLF * h + 1024 * (g + 1)],
                        qp[:],
                    )

            for h in range(NHF):
                he = h_halves[h]
                hv = he[:].rearrange("p (c l) -> p c l", l=EXT)
                et = etm.tile([P, HLF], BF16, tag="etm")
                etv = et[:].rearrange("p (c l) -> p c l", l=128)
                if h == NHF - 1:
                    # split the last exp: halves the gram tail on the
                    # boundary critical path
                    nc.scalar.activation(etv[:, 0:8], hv[:, 0:8, 0:128],
                                         AF.Exp)
                    nc.scalar.activation(etv[:, 8:16], hv[:, 8:16, 0:128],
                                         AF.Exp)
                else:
                    nc.scalar.activation(etv, hv[:, :, 0:128], AF.Exp)
                et_tiles.append(et)
                for c in range(HLF // 128):
                    nc.tensor.matmul(
                        G_ps[:],
                        et[:, 128 * c:128 * (c + 1)],
                        he[:, EXT * c:EXT * (c + 1)],
                        start=(h == 0 and c == 0),
                        stop=(h == NHF - 1 and c == HLF // 128 - 1),
                    )
                if blk == 0 and h + 2 < NHF:
                    ingest_half(h + 2)
                # early q-glue: the GPSIMD scale chain starts as soon as
                # exps land; transposes placed where PE has slack
                if h == 3:
                    q_scale(0)
                    q_xpose(0)

            # ======================= M1 build (before q glue: keeps the
            # M1u matmul ahead of the q transposes in the PE queue; high
            # priority so the DVE steps jump the q-glue queue) ===========
            with tc.high_priority():
                ksC = smal.tile([P, 1], F32, tag="ksC")
                nc.vector.reciprocal(ksC[:], G_ps[:, 128:129])
                G_sb = smal.tile([P, P], BF16, tag="Gsb")
                nc.vector.tensor_tensor(G_sb[:], G_ps[:, 0:128], headmask,
                                        op=MUL)
                # block-diag transpose: DVE 32x32 block transpose (off-diag
                # blocks are zero)
                GT_sb = smal.tile([P, P], BF16, tag="gtsb")
                nc.vector.transpose(GT_sb[:], G_sb[:])
                M1u_t = gps.tile([P, P], F32, tag="G")
                nc.tensor.matmul(M1u_t[:], GT_sb[:], W1bd[blk])
                M1 = smal.tile([P, P], BF16, tag="m1")
                nc.vector.tensor_scalar_mul(M1[:], M1u_t[:], ksC[:])

            # ======================= phase 2 ============================
            he_next = ([new_he_half(blk + 1, h) for h in range(NHF)]
                       if not last else None)

            def z1_mm(t):
                z1 = z1ps.tile([P, 1024], F32, tag="z1", name=f"z1_{blk}_{t}")
                for hh in range(2):
                    nc.tensor.matmul(
                        z1[:, 512 * hh:512 * (hh + 1)], M1[:],
                        q_cm[:, 1024 * t + 512 * hh:1024 * t + 512 * (hh + 1)])
                return z1

            def g1_act(t, z1):
                a1 = a1p.tile([P, 1024], BF16, tag="a1")
                nc.scalar.activation(a1[:], z1[:], AF.Gelu, bias=b1rep[blk])
                return a1

            def z2_mm(t, a1):
                # two [128,512] halves, 2-buf: PE fills half h+1 while the
                # ACT gelu reads half h — kills the serial z2->g2 chain
                halves = []
                for hh in range(2):
                    z2 = z2ps.tile([P, 512], F32, tag="z2",
                                   name=f"z2_{blk}_{t}_{hh}")
                    if last:
                        nc.tensor.matmul(
                            z2[:], W2bd[blk],
                            a1[:, 512 * hh:512 * (hh + 1)])
                    else:
                        # bias preload (K=1 rank-1) + a1-stationary chunks
                        nc.tensor.matmul(
                            z2[:], onesrow, b2row4[blk],
                            start=True, stop=False)
                        for cc in range(4):
                            cch = 4 * hh + cc
                            nc.tensor.matmul(
                                z2[:, 128 * cc:128 * (cc + 1)],
                                a1[:, 128 * cch:128 * (cch + 1)],
                                W2bd[blk],
                                start=False, stop=(cc == 3))
                    halves.append(z2)
                return halves

            def g2_act(t, z2h):
                for hh, z2 in enumerate(z2h):
                    if last:
                        hn = a1p.tile([P, 512], BF16, tag="a1")
                        nc.scalar.activation(
                            hn[:], z2[:], AF.Gelu, bias=b2rep[blk])
                        nc.vector.reduce_sum(
                            pooled_parts[:, 2 * t + hh:2 * t + hh + 1],
                            hn[:], axis=mybir.AxisListType.X)
                    else:
                        # token-major write into next block's he tiles
                        he = he_next[t // 2]
                        hv = he[:].rearrange("p (c l) -> p c l", l=EXT)
                        cb = 8 * (t % 2) + 4 * hh
                        nc.scalar.activation(
                            hv[:, cb:cb + 4, 0:128],
                            z2[:].rearrange("p (c l) -> p c l", l=128),
                            AF.Gelu)

            # interleaved schedule: q-glue emitted per slice just before the
            # phase-2 tiles that consume it (PE queue is in-order), and
            # z1(t+1)/g1(t+1) emitted before g2(t) so ACT never waits on
            # the z2 matmuls.
            prev = None
            for t in range(NTP):
                z1_t = z1_mm(t)
                a1_t = g1_act(t, z1_t)
                if prev is not None:
                    pt, pa = prev
                    z2_p = z2_mm(pt, pa)
                    g2_act(pt, z2_p)
                prev = (t, a1_t)
                # q-glue for later halves, in PE slack after a z2/g2
                # pair; half h must land before z1(tp 2h)
                if t == 0:
                    q_scale(1)
                    q_xpose(1)
                elif t % 2 == 1 and (t + 3) // 2 < NHF:
                    q_scale((t + 3) // 2)
                    q_xpose((t + 3) // 2)
            pt, pa = prev
            z2_p = z2_mm(pt, pa)
            g2_act(pt, z2_p)

            if not last:
                h_halves = he_next

        # ============================ head ==============================
        psum_ = smal.tile([P, 1], F32, tag="poolsum")
        nc.vector.reduce_sum(psum_[:], pooled_parts[:],
                             axis=mybir.AxisListType.X)
        y_t = gps.tile([P, 1], F32, tag="G")
        for b in range(B_LOC):
            sl = slice(C * b, C * (b + 1))
            nc.tensor.matmul(
                y_t[sl, :], Whrep[sl, :], psum_[sl, :],
                tile_position=(C * b, C * b),
            )
        ybn = smal.tile([P, 1], F32, tag="ybn")
        nc.vector.tensor_scalar(
            ybn[:], y_t[:], svecL, tvec, op0=MUL, op1=ADD,
        )
        yg = smal.tile([P, 1], F32, tag="yg")
        nc.scalar.activation(yg[:], ybn[:], AF.Gelu)
        o_t = gps.tile([P, 1], F32, tag="G")
        for b in range(B_LOC):
            nc.tensor.matmul(
                o_t[C * b:C * b + DOUT, :],
                Wfrep[C * b:C * (b + 1), :],
                yg[C * b:C * (b + 1), :],
                tile_position=(C * b, C * b),
            )
        ob = smal.tile([P, C], F32, tag="ob")
        for b in range(B_LOC):
            sl = slice(C * b, C * b + DOUT)
            nc.vector.tensor_tensor(ob[sl, 0:1], o_t[sl, :], bf_s[sl, :],
                                    op=ADD)
        obT = smal.tile([P, C], F32, tag="obT")
        nc.vector.transpose(obT[:], ob[:])
        obv = obT[:].rearrange("(b r) d -> b r d", b=B_LOC)[:, 0, 0:DOUT]
        nc.sync.dma_start(out_d[:, :], obv)

    _fix_sync_waits(nc)
    from concourse.library_overlay import lower_extended_insts
    lower_extended_insts(nc)   # encode custom-DVE InstISA bytes
    return nc


# ------------------------------------------------------------- host pack --
def _bf16_bits(a):
    """float32 -> bf16 bit pattern (round to nearest even), as uint16."""
    u = np.ascontiguousarray(a, dtype="<f4").view("<u4")
    r = ((u >> 16) & 1) + np.uint32(0x7FFF)
    return ((u + r) >> 16).astype("<u2")


def _rep4(v):
    """[32]/[32,k] -> [128]/[128,k] replicated across 4 batch strips."""
    return np.concatenate([v] * B_LOC, axis=0)


def pack_weights(a):
    """Build the two packed weight tensors from the raw input dict."""
    bf = np.zeros((P, NBF), np.float32)
    eye4 = np.eye(B_LOC, dtype=np.float32)
    for i in range(NB):
        bf[:, 128 * i:128 * (i + 1)] = np.kron(eye4, a["fcW1"][i])
        bf[:, 384 + 128 * i:384 + 128 * (i + 1)] = np.kron(eye4, a["fcW2"][i])
    idx = np.arange(P)
    bf[:, 768:896] = (idx[:, None] // DH == idx[None, :] // DH)
    bf[:, 896] = 1.0
    bf[0, 898:1026] = 1.0                       # ones row
    for i in range(NB - 1):
        bf[0, 1026 + 512 * i:1026 + 512 * (i + 1)] = np.tile(
            _rep4(a["fcb2"][i]), 4)             # b2 pattern x4 (512 cols)
    bfu = _bf16_bits(bf).view("<u4").reshape(P, NBF // 2)

    f32 = np.zeros((P, NF32), np.float32)
    for i in range(NB):
        f32[:, i] = _rep4(a["fcb1"][i])
        f32[:, 3 + i] = _rep4(a["fcb2"][i])
    svec = a["bn_gamma"] / np.sqrt(a["bn_var"] + BN_EPS)
    f32[:, 6] = _rep4(svec / L)
    f32[:, 7] = _rep4((a["bh"] - a["bn_mean"]) * svec + a["bn_beta"])
    f32[:, 8:40] = _rep4(a["Wh"])
    f32[:, 40:50] = _rep4(a["Wf"])
    bfcol = np.zeros(C, np.float32)
    bfcol[:DOUT] = a["bf"]
    f32[:, 50] = _rep4(bfcol)
    return np.ascontiguousarray(bfu), np.ascontiguousarray(f32)


_NC_CACHE = [None]


def kernel(**inputs) -> np.ndarray:
    arrs = {k: np.asarray(v, dtype=np.float32) for k, v in inputs.items()}
    x = arrs["x"]
    B = x.shape[0]
    n_cores = 8
    bl = B // n_cores

    if _NC_CACHE[0] is None:
        _NC_CACHE[0] = build_program()
    nc = _NC_CACHE[0]

    wbf, wf32 = pack_weights(arrs)
    in_maps = [
        {"x": np.ascontiguousarray(x[bl * i: bl * (i + 1)]),
         "wbf": wbf, "wf32": wf32}
        for i in range(n_cores)
    ]
    res = run_bass_kernel_spmd(nc, in_maps, list(range(n_cores))).results
    return np.concatenate([res[i]["out"] for i in range(n_cores)], axis=0)
